# revision 18
# baseline (speedup 1.0000x reference)
"""Dual attention (DANet-style spatial + channel attention) on 8 Trainium2
NeuronCores.

Sharding: data-parallel over batch B=4, and each batch's output positions
(m in [0, 4096)) split in half across 2 cores -> 8 identical single-core
programs, no collectives. Each core receives its batch's full x (for k/v and
the channel-attention statistics) plus the m-slice of x it owns (for q and
the residual), and produces out[:, m_slice].

Per-core math (x: [512, 4096], m-chunk: 2048 positions):
  spatial:  q=Wq@xq+bq; k=Wk@x+bk; E^T[n,m]=k[:,n].q[:,m]; P=exp(E^T)
            (no max subtraction -- |E| < ~60 so exp fits fp32/bf16 range);
            vT[n,c]=(Wv@x+bv)^T; U[c,m]=sum_n vT[n,c]P[n,m]; Z[m]=sum_n P[n,m]
            s_out = U/Z;  spatial = gamma_s*s_out + xq
  channel:  pT[n,d]=(Wd@x+bd)^T; e=pT^T@pT; c_attn=softmax(rowmax(e)-e);
            c2=gamma_c*(c_attn@p)[:,m]+p[:,m]; channel = Wu@c2+bu
  out = spatial + channel

Performance structure:
  - energy computed TRANSPOSED (n on partitions): exp and the U/Z matmuls
    consume it directly, no [2048,4096] transposes anywhere.
  - fp16 matmuls (1 PE cycle/row; host converts x/weights), bf16 for the
    dominant U matmul (P=exp(E) can reach ~1e24, beyond fp16 range).
    PSUM accumulation is always fp32. fp32 residual path keeps the output
    accurate: ~6e-4 scale-relative absmax vs the fp32 reference.
  - engine split: PE matmuls; ACT exp + bias-adds; DVE softmax-denominator
    accumulation and final combines. Channel output + residual (R) are
    precomputed before the main loop so the per-chunk epilogue is short.
"""
import sys

sys.path.insert(0, '/opt/trn_rl_repo')

import numpy as np

import concourse.bass as bass
import concourse.tile as tile
from concourse import bacc, bass_utils, mybir
from concourse.masks import make_identity

# Problem shapes (fixed by the task spec)
B, C, WIDTH, HEIGHT = 4, 512, 64, 64
N = WIDTH * HEIGHT      # 4096 spatial positions
DK = 64                 # attention inner dim (and channel-attn dim)
NCORES = 8
M = N // 2              # 2048 output positions per core
P = 128
KC = C // P             # 4 input-channel chunks
NT = N // P             # 32 key-position tiles
FREE = 512              # matmul moving free dim (one PSUM bank of fp32)
MCH = M // FREE         # 4 m-chunks per core
CCH = C // P            # 4 output-channel chunks

F32 = mybir.dt.float32
F16 = mybir.dt.float16
BF16 = mybir.dt.bfloat16
AX = mybir.AxisListType
ALU = mybir.AluOpType
ACTF = mybir.ActivationFunctionType


def _bcast_dram(ap, nparts):
    """AP reading a [1]-ish DRAM tensor broadcast across nparts partitions."""
    return bass.AP(tensor=ap.tensor, offset=ap.offset,
                   ap=[[0, nparts], *ap.ap])


def _build_program(tc, io):
    nc = tc.nc
    x_d, xq_d, xqh_d = io['x'], io['xq'], io['xqh']
    out_d = io['out']

    const_cm = tc.tile_pool(name='const', bufs=1)
    const = const_cm.__enter__()

    # ---- persistent SBUF tensors (DMAs issued in consumption order) ----
    xqh_sb = const.tile([P, KC, M], F16)   # fp16 matmul operand (first user)
    xqh_r = xqh_d.rearrange("(kc p) m -> p kc m", p=P)
    for mq in range(2):
        qsl = slice(mq * (M // 2), (mq + 1) * (M // 2))
        nc.sync.dma_start(xqh_sb[:, :, qsl], xqh_r[:, :, qsl])
    wq_sb = const.tile([P, KC, DK], F16)
    nc.sync.dma_start(wq_sb[:], io['wqT'].rearrange("(kc p) d -> p kc d", p=P))
    wd_sb = const.tile([P, KC, DK], F16)
    nc.sync.dma_start(wd_sb[:], io['wdT'].rearrange("(kc p) d -> p kc d", p=P))
    wk_sb = const.tile([P, KC, DK], F16)
    nc.sync.dma_start(wk_sb[:], io['wkT'].rearrange("(kc p) d -> p kc d", p=P))
    wv_sb = const.tile([P, KC, C], F16)
    nc.sync.dma_start(wv_sb[:], io['wvT'].rearrange("(kc p) c -> p kc c", p=P))
    wu_sb = const.tile([DK, C], F16)
    nc.sync.dma_start(wu_sb[:], io['wuT'][:])

    bq_sb = const.tile([DK, 1], F32)
    nc.sync.dma_start(bq_sb[:], io['bq'][:, None])
    bk_sb = const.tile([DK, 1], F32)
    nc.sync.dma_start(bk_sb[:], io['bk'][:, None])
    bd_sb = const.tile([DK, 1], F32)
    nc.sync.dma_start(bd_sb[:], io['bd'][:, None])
    bdrow_sb = const.tile([1, DK], F16)    # bd as a row, for the pT rank-1 add
    nc.sync.dma_start(bdrow_sb[:], io['bdh'][None, :])
    bvrow_sb = const.tile([1, C], F32)
    nc.sync.dma_start(bvrow_sb[:], io['bv'][None, :])
    bu_sb = const.tile([P, CCH], F32)
    nc.sync.dma_start(bu_sb[:], io['bu'].rearrange("(kc p) -> p kc", p=P))
    gs_sb = const.tile([1, 1], F32)
    nc.sync.dma_start(gs_sb[:], io['gs'][:, None])
    gc_sb = const.tile([DK, 1], F32)
    nc.sync.dma_start(gc_sb[:], _bcast_dram(io['gc'][:, None], DK))

    ones_row32 = const.tile([1, P], F32)   # lhsT for fp32 partition-broadcast
    nc.vector.memset(ones_row32[:], 1.0)
    ones_row16 = const.tile([1, P], F16)   # lhsT for fp16 rank-1 bias adds
    nc.vector.memset(ones_row16[:], 1.0)
    ones_colb = const.tile([P, 1], BF16)   # lhsT for bf16 partition-sum
    nc.vector.memset(ones_colb[:], 1.0)
    ones_rowb = const.tile([1, P], BF16)   # lhsT for bf16 partition-broadcast
    nc.vector.memset(ones_rowb[:], 1.0)
    ident16 = const.tile([DK, DK], F16)    # for the tiny c_attn transpose
    make_identity(nc, ident16[:])

    k_sb = const.tile([DK, N], F16)        # keys,   [d, n]
    q_sb = const.tile([DK, M], F16)        # queries,[d, m]
    pc_sb = const.tile([DK, M], F16)       # channel proj on the m-slice
    pT_sb = const.tile([P, NT, DK], F16)   # channel proj transposed [n, nt, d]
    vT_sb = const.tile([P, NT, C], BF16)   # values transposed, [n, nt, c]
    c2_sb = const.tile([DK, M], F16)       # gamma_c * c_attn@p + p on m-slice
    bvb_sb = const.tile([P, C], F32)       # bv broadcast to all 128 partitions
    r_sb = const.tile([P, CCH, M], F32)    # channel-out + xq residual

    # ---- phase 1a: q/pc from xqh (PE starts while the big x DMA runs) ----
    with tc.tile_pool(name='ps0', bufs=2, space='PSUM') as ps0:
        for j in range(M // FREE):
            sl = slice(j * FREE, (j + 1) * FREE)
            pq = ps0.tile([DK, FREE], F32, tag='pq')
            for kc in range(KC):
                nc.tensor.matmul(pq[:], lhsT=wq_sb[:, kc],
                                 rhs=xqh_sb[:, kc, sl],
                                 start=(kc == 0), stop=(kc == KC - 1))
            nc.scalar.activation(q_sb[:, sl], pq[:], ACTF.Identity,
                                 bias=bq_sb[:])
            ppc = ps0.tile([DK, FREE], F32, tag='pq')
            for kc in range(KC):
                nc.tensor.matmul(ppc[:], lhsT=wd_sb[:, kc],
                                 rhs=xqh_sb[:, kc, sl],
                                 start=(kc == 0), stop=(kc == KC - 1))
            nc.scalar.activation(pc_sb[:, sl], ppc[:], ACTF.Identity,
                                 bias=bd_sb[:])

    # ---- phase 1b: everything that needs the full x ----
    with tc.tile_pool(name='xp', bufs=1) as xp, \
         tc.tile_pool(name='ps1', bufs=2, space='PSUM') as ps1, \
         tc.tile_pool(name='ps1s', bufs=2, space='PSUM') as ps1s, \
         tc.tile_pool(name='ps1e', bufs=1, space='PSUM') as ps1e:
        x_sb = xp.tile([P, KC, N], F16)
        x_r = x_d.rearrange("(kc p) n -> p kc n", p=P)
        for nq in range(8):
            qsl = slice(nq * (N // 8), (nq + 1) * (N // 8))
            nc.sync.dma_start(x_sb[:, :, qsl], x_r[:, :, qsl])
        xq_sb = xp.tile([P, KC, M], F32)   # fp32 residual source
        nc.sync.dma_start(xq_sb[:], xq_d.rearrange("(kc p) m -> p kc m", p=P))

        # bv broadcast to [128, C] once
        nc.gpsimd.partition_broadcast(bvb_sb[:], bvrow_sb[:], channels=P)

        # k = Wk@x + bk over the full N
        for j in range(N // FREE):
            sl = slice(j * FREE, (j + 1) * FREE)
            pk = ps1.tile([DK, FREE], F32, tag='pk')
            for kc in range(KC):
                nc.tensor.matmul(pk[:], lhsT=wk_sb[:, kc],
                                 rhs=x_sb[:, kc, sl],
                                 start=(kc == 0), stop=(kc == KC - 1))
            nc.scalar.activation(k_sb[:, sl], pk[:], ACTF.Identity,
                                 bias=bk_sb[:])

        # vT and pT tiles: [n, c] = sum_kc x[kc, n]^T W^T[kc, c]  (+ bias)
        for nt in range(NT):
            nsl = slice(nt * P, (nt + 1) * P)
            pv = ps1.tile([P, C], F32, tag='pv')
            for kc in range(KC):
                nc.tensor.matmul(pv[:], lhsT=x_sb[:, kc, nsl],
                                 rhs=wv_sb[:, kc],
                                 start=(kc == 0), stop=(kc == KC - 1))
            nc.vector.tensor_add(vT_sb[:, nt], in0=pv[:], in1=bvb_sb[:])

            pt_ps = ps1s.tile([P, DK], F32, tag='ptp')
            for kc in range(KC):
                nc.tensor.matmul(pt_ps[:], lhsT=x_sb[:, kc, nsl],
                                 rhs=wd_sb[:, kc],
                                 start=(kc == 0), stop=False)
            nc.tensor.matmul(pt_ps[:], lhsT=ones_row16[:], rhs=bdrow_sb[:],
                             start=False, stop=True)
            nc.vector.tensor_copy(pT_sb[:, nt], pt_ps[:])

        # ---- channel attention (tiny): e = pT^T @ pT, softmax, c2, R ----
        with tc.tile_pool(name='sb2', bufs=2) as sb2:
            e_ps = ps1e.tile([DK, DK], F32, tag='e')
            for nt in range(NT):
                nc.tensor.matmul(e_ps[:], lhsT=pT_sb[:, nt], rhs=pT_sb[:, nt],
                                 start=(nt == 0), stop=(nt == NT - 1))
            # c_attn = softmax(rowmax(e) - e) == exp(rowmin(e) - e) / rowsum
            e_sb = sb2.tile([DK, DK], F32, tag='e')
            nc.vector.tensor_copy(e_sb[:], e_ps[:])
            mn_sb = sb2.tile([DK, 1], F32, tag='mn')
            nc.vector.tensor_reduce(mn_sb[:], e_sb[:], axis=AX.X, op=ALU.min)
            h_sb = sb2.tile([DK, DK], F32, tag='h')
            nc.scalar.activation(h_sb[:], e_sb[:], ACTF.Exp,
                                 bias=mn_sb[:], scale=-1.0)
            zc_sb = sb2.tile([DK, 1], F32, tag='zc')
            nc.vector.tensor_reduce(zc_sb[:], h_sb[:], axis=AX.X, op=ALU.add)
            nc.vector.reciprocal(zc_sb[:], zc_sb[:])
            cat16_sb = sb2.tile([DK, DK], F16, tag='cat16')
            nc.vector.tensor_scalar_mul(cat16_sb[:], in0=h_sb[:],
                                        scalar1=zc_sb[:])
            catT_ps = ps1e.tile([DK, DK], F16, tag='catp')
            nc.tensor.transpose(catT_ps[:], cat16_sb[:], ident16[:])
            catT_sb = sb2.tile([DK, DK], F16, tag='cat')
            nc.vector.tensor_copy(catT_sb[:], catT_ps[:])

            # c2 = gamma_c * (c_attn @ p)[:, m_slice] + pc
            for j in range(M // FREE):
                sl = slice(j * FREE, (j + 1) * FREE)
                co_ps = ps1.tile([DK, FREE], F32, tag='pk')
                nc.tensor.matmul(co_ps[:], lhsT=catT_sb[:], rhs=pc_sb[:, sl],
                                 start=True, stop=True)
                nc.vector.scalar_tensor_tensor(
                    out=c2_sb[:, sl], in0=co_ps[:], scalar=gc_sb[:],
                    in1=pc_sb[:, sl], op0=ALU.mult, op1=ALU.add)

            # R = Wu@c2 + bu + xq  (the whole non-spatial part of the output)
            for mc in range(MCH):
                msl = slice(mc * FREE, (mc + 1) * FREE)
                for cc in range(CCH):
                    wu_ps = ps1.tile([P, FREE], F32, tag='pv')
                    nc.tensor.matmul(wu_ps[:],
                                     lhsT=wu_sb[:, cc * P:(cc + 1) * P],
                                     rhs=c2_sb[:, msl], start=True, stop=True)
                    cob_sb = sb2.tile([P, FREE], F32, tag='cob')
                    nc.scalar.activation(cob_sb[:], wu_ps[:], ACTF.Identity,
                                         bias=bu_sb[:, cc:cc + 1])
                    nc.vector.tensor_add(r_sb[:, cc, msl], in0=cob_sb[:],
                                         in1=xq_sb[:, cc, msl])

    # ---- main loop: E^T -> exp -> U/Z accumulation, one m-chunk at a time ----
    out_r = out_d.rearrange("(kc p) m -> p kc m", p=P)
    with tc.tile_pool(name='upool', bufs=4, space='PSUM') as upool, \
         tc.tile_pool(name='epool', bufs=3, space='PSUM') as epool, \
         tc.tile_pool(name='zpool', bufs=1, space='PSUM') as zpool, \
         tc.tile_pool(name='pt', bufs=6) as ptp, \
         tc.tile_pool(name='ssb', bufs=2) as ssb, \
         tc.tile_pool(name='ot', bufs=4) as otp:
        for mc in range(MCH):
            msl = slice(mc * FREE, (mc + 1) * FREE)
            u_ps = [upool.tile([P, FREE], F32, tag='u', name=f'u{mc}_{i}')
                    for i in range(CCH)]
            s_sb = ssb.tile([P, FREE], BF16, tag='s')
            for nt in range(NT):
                nsl = slice(nt * P, (nt + 1) * P)
                e_t = epool.tile([P, FREE], F32, tag='et')
                nc.tensor.matmul(e_t[:], lhsT=k_sb[:, nsl],
                                 rhs=q_sb[:, msl], start=True, stop=True)
                p_t = ptp.tile([P, FREE], BF16, tag='p')
                nc.scalar.activation(p_t[:], e_t[:], ACTF.Exp)
                if nt == 0:
                    nc.vector.tensor_copy(s_sb[:], p_t[:])
                else:
                    nc.vector.tensor_add(s_sb[:], in0=s_sb[:], in1=p_t[:])
                for cc in range(CCH):
                    nc.tensor.matmul(u_ps[cc][:],
                                     lhsT=vT_sb[:, nt, cc * P:(cc + 1) * P],
                                     rhs=p_t[:],
                                     start=(nt == 0), stop=(nt == NT - 1))
            # Z = colsum(S); Zb = gamma_s / Z broadcast to 128 partitions
            z_ps = zpool.tile([1, FREE], F32, tag='z')
            nc.tensor.matmul(z_ps[:], lhsT=ones_colb[:], rhs=s_sb[:],
                             start=True, stop=True)
            zr_sb = ssb.tile([1, FREE], F32, tag='zr')
            nc.vector.reciprocal_approx_fast(out=zr_sb[:], in_=z_ps[:])
            zrb_sb = ssb.tile([1, FREE], BF16, tag='zrb')
            nc.vector.tensor_scalar_mul(zrb_sb[:], in0=zr_sb[:], scalar1=gs_sb[:])
            zb_ps = zpool.tile([P, FREE], F32, tag='z')
            nc.tensor.matmul(zb_ps[:], lhsT=ones_rowb[:], rhs=zrb_sb[:],
                             start=True, stop=True)
            zb_sb = ssb.tile([P, FREE], F32, tag='zb')
            nc.vector.tensor_copy(zb_sb[:], zb_ps[:])
            # combine: out = U*(gamma_s/Z) + R, then store
            for cc in range(CCH):
                o_sb = otp.tile([P, FREE], F32, tag='o')
                nc.vector.tensor_tensor(o_sb[:], u_ps[cc][:], zb_sb[:], ALU.mult)
                nc.vector.tensor_add(o_sb[:], in0=o_sb[:], in1=r_sb[:, cc, msl])
                nc.sync.dma_start(out_r[:, cc, msl], o_sb[:])

    const_cm.__exit__(None, None, None)


_CACHE = {}


def _get_compiled():
    if 'nc' in _CACHE:
        return _CACHE['nc']
    nc = bacc.Bacc("TRN2", num_devices=NCORES)
    io = {
        'x': nc.dram_tensor('x', [C, N], F16, kind='ExternalInput').ap(),
        'xq': nc.dram_tensor('xq', [C, M], F32, kind='ExternalInput').ap(),
        'xqh': nc.dram_tensor('xqh', [C, M], F16, kind='ExternalInput').ap(),
        'wqT': nc.dram_tensor('wqT', [C, DK], F16, kind='ExternalInput').ap(),
        'wkT': nc.dram_tensor('wkT', [C, DK], F16, kind='ExternalInput').ap(),
        'wvT': nc.dram_tensor('wvT', [C, C], F16, kind='ExternalInput').ap(),
        'wdT': nc.dram_tensor('wdT', [C, DK], F16, kind='ExternalInput').ap(),
        'wuT': nc.dram_tensor('wuT', [DK, C], F16, kind='ExternalInput').ap(),
        'bq': nc.dram_tensor('bq', [DK], F32, kind='ExternalInput').ap(),
        'bk': nc.dram_tensor('bk', [DK], F32, kind='ExternalInput').ap(),
        'bd': nc.dram_tensor('bd', [DK], F32, kind='ExternalInput').ap(),
        'bdh': nc.dram_tensor('bdh', [DK], F16, kind='ExternalInput').ap(),
        'bv': nc.dram_tensor('bv', [C], F32, kind='ExternalInput').ap(),
        'bu': nc.dram_tensor('bu', [C], F32, kind='ExternalInput').ap(),
        'gs': nc.dram_tensor('gs', [1], F32, kind='ExternalInput').ap(),
        'gc': nc.dram_tensor('gc', [1], F32, kind='ExternalInput').ap(),
        'out': nc.dram_tensor('out', [C, M], F32, kind='ExternalOutput').ap(),
    }
    with tile.TileContext(nc) as tc:
        _build_program(tc, io)
    nc.compile()
    _CACHE['nc'] = nc
    return nc


def make_in_maps(x, Wq, bq, Wk, bk, Wv, bv, gamma_s, Wd, bd, Wu, bu, gamma_c):
    """Build the 8 per-core input dicts from the full problem inputs."""
    f32 = lambda a: np.ascontiguousarray(np.asarray(a, dtype=np.float32))
    f16 = lambda a: np.ascontiguousarray(np.asarray(a, dtype=np.float32)
                                         .astype(np.float16))
    x = f32(x).reshape(B, C, N)
    shared = {
        'wqT': f16(np.asarray(Wq).T), 'wkT': f16(np.asarray(Wk).T),
        'wvT': f16(np.asarray(Wv).T), 'wdT': f16(np.asarray(Wd).T),
        'wuT': f16(np.asarray(Wu).T),
        'bq': f32(bq), 'bk': f32(bk), 'bd': f32(bd), 'bdh': f16(bd),
        'bv': f32(bv), 'bu': f32(bu), 'gs': f32(gamma_s), 'gc': f32(gamma_c),
    }
    in_maps = []
    for core in range(NCORES):
        b, h = divmod(core, 2)
        xq = x[b][:, h * M:(h + 1) * M]
        in_maps.append({
            'x': f16(x[b]),
            'xq': f32(xq),
            'xqh': f16(xq),
            **shared,
        })
    return in_maps


def assemble_out(results):
    """Stitch the 8 per-core [C, M] outputs back to [B, C, W, H]."""
    full = np.empty((B, C, N), np.float32)
    for core, res in enumerate(results):
        b, h = divmod(core, 2)
        full[b][:, h * M:(h + 1) * M] = res['out']
    return full.reshape(B, C, WIDTH, HEIGHT)


def kernel(**inputs):
    nc = _get_compiled()
    in_maps = make_in_maps(**inputs)
    res = bass_utils.run_bass_kernel_spmd(nc, in_maps, core_ids=list(range(NCORES)))
    return assemble_out(res.results)


# revision 19
# speedup vs baseline: 1.0163x; 1.0163x over previous
"""Dual attention (DANet-style spatial + channel attention) on 8 Trainium2
NeuronCores.

Sharding: data-parallel over batch B=4, and each batch's output positions
(m in [0, 4096)) split in half across 2 cores -> 8 identical single-core
programs, no collectives. Each core receives its batch's full x (for k/v and
the channel-attention statistics) plus the m-slice of x it owns (for q and
the residual), and produces out[:, m_slice].

Per-core math (x: [512, 4096], m-chunk: 2048 positions):
  spatial:  q=Wq@xq+bq; k=Wk@x+bk; E^T[n,m]=k[:,n].q[:,m]; P=exp(E^T)
            (no max subtraction -- |E| < ~60 so exp fits fp32/bf16 range);
            vT[n,c]=(Wv@x+bv)^T; U[c,m]=sum_n vT[n,c]P[n,m]; Z[m]=sum_n P[n,m]
            s_out = U/Z;  spatial = gamma_s*s_out + xq
  channel:  pT[n,d]=(Wd@x+bd)^T; e=pT^T@pT; c_attn=softmax(rowmax(e)-e);
            c2=gamma_c*(c_attn@p)[:,m]+p[:,m]; channel = Wu@c2+bu
  out = spatial + channel

Performance structure:
  - energy computed TRANSPOSED (n on partitions): exp and the U/Z matmuls
    consume it directly, no [2048,4096] transposes anywhere.
  - fp16 matmuls (1 PE cycle/row; host converts x/weights), bf16 for the
    dominant U matmul (P=exp(E) can reach ~1e24, beyond fp16 range).
    PSUM accumulation is always fp32. fp32 residual path keeps the output
    accurate: ~6e-4 scale-relative absmax vs the fp32 reference.
  - engine split: PE matmuls; ACT exp + bias-adds; DVE softmax-denominator
    accumulation and final combines. Channel output + residual (R) are
    precomputed before the main loop so the per-chunk epilogue is short.
"""
import sys

sys.path.insert(0, '/opt/trn_rl_repo')

import numpy as np

import concourse.bass as bass
import concourse.tile as tile
from concourse import bacc, bass_utils, mybir
from concourse.masks import make_identity

# Problem shapes (fixed by the task spec)
B, C, WIDTH, HEIGHT = 4, 512, 64, 64
N = WIDTH * HEIGHT      # 4096 spatial positions
DK = 64                 # attention inner dim (and channel-attn dim)
NCORES = 8
M = N // 2              # 2048 output positions per core
P = 128
KC = C // P             # 4 input-channel chunks
NT = N // P             # 32 key-position tiles
FREE = 512              # matmul moving free dim (one PSUM bank of fp32)
MCH = M // FREE         # 4 m-chunks per core
CCH = C // P            # 4 output-channel chunks

F32 = mybir.dt.float32
F16 = mybir.dt.float16
BF16 = mybir.dt.bfloat16
AX = mybir.AxisListType
ALU = mybir.AluOpType
ACTF = mybir.ActivationFunctionType


def _bcast_dram(ap, nparts):
    """AP reading a [1]-ish DRAM tensor broadcast across nparts partitions."""
    return bass.AP(tensor=ap.tensor, offset=ap.offset,
                   ap=[[0, nparts], *ap.ap])


def _build_program(tc, io):
    nc = tc.nc
    x_d, xq_d, xqh_d = io['x'], io['xq'], io['xqh']
    out_d = io['out']

    const_cm = tc.tile_pool(name='const', bufs=1)
    const = const_cm.__enter__()

    # ---- persistent SBUF tensors (DMAs issued in consumption order) ----
    xqh_sb = const.tile([P, KC, M], F16)   # fp16 matmul operand (first user)
    xqh_r = xqh_d.rearrange("(kc p) m -> p kc m", p=P)
    for mq in range(2):
        qsl = slice(mq * (M // 2), (mq + 1) * (M // 2))
        nc.sync.dma_start(xqh_sb[:, :, qsl], xqh_r[:, :, qsl])
    wq_sb = const.tile([P, KC, DK], F16)
    nc.sync.dma_start(wq_sb[:], io['wqT'].rearrange("(kc p) d -> p kc d", p=P))
    wd_sb = const.tile([P, KC, DK], F16)
    nc.sync.dma_start(wd_sb[:], io['wdT'].rearrange("(kc p) d -> p kc d", p=P))
    wk_sb = const.tile([P, KC, DK], F16)
    nc.sync.dma_start(wk_sb[:], io['wkT'].rearrange("(kc p) d -> p kc d", p=P))
    wv_sb = const.tile([P, KC, C], F16)
    nc.sync.dma_start(wv_sb[:], io['wvT'].rearrange("(kc p) c -> p kc c", p=P))
    wu_sb = const.tile([DK, C], F16)
    nc.sync.dma_start(wu_sb[:], io['wuT'][:])

    bq_sb = const.tile([DK, 1], F32)
    nc.sync.dma_start(bq_sb[:], io['bq'][:, None])
    bk_sb = const.tile([DK, 1], F32)
    nc.sync.dma_start(bk_sb[:], io['bk'][:, None])
    bd_sb = const.tile([DK, 1], F32)
    nc.sync.dma_start(bd_sb[:], io['bd'][:, None])
    bdrow_sb = const.tile([1, DK], F16)    # bd as a row, for the pT rank-1 add
    nc.sync.dma_start(bdrow_sb[:], io['bdh'][None, :])
    bvrow_sb = const.tile([1, C], F32)
    nc.sync.dma_start(bvrow_sb[:], io['bv'][None, :])
    bu_sb = const.tile([P, CCH], F32)
    nc.sync.dma_start(bu_sb[:], io['bu'].rearrange("(kc p) -> p kc", p=P))
    gs_sb = const.tile([1, 1], F32)
    nc.sync.dma_start(gs_sb[:], io['gs'][:, None])
    gc_sb = const.tile([DK, 1], F32)
    nc.sync.dma_start(gc_sb[:], _bcast_dram(io['gc'][:, None], DK))

    ones_row16 = const.tile([1, P], F16)   # lhsT for fp16 rank-1 bias adds
    nc.vector.memset(ones_row16[:], 1.0)
    ones_colb = const.tile([P, 1], BF16)   # lhsT for bf16 partition-sum
    nc.vector.memset(ones_colb[:], 1.0)
    ones_rowb = const.tile([1, P], BF16)   # lhsT for bf16 partition-broadcast
    nc.vector.memset(ones_rowb[:], 1.0)
    ident16 = const.tile([DK, DK], F16)    # for the tiny c_attn transpose
    make_identity(nc, ident16[:])

    k_sb = const.tile([DK, N], F16)        # keys,   [d, n]
    q_sb = const.tile([DK, M], F16)        # queries,[d, m]
    pc_sb = const.tile([DK, M], F16)       # channel proj on the m-slice
    pT_sb = const.tile([P, NT, DK], F16)   # channel proj transposed [n, nt, d]
    vT_sb = const.tile([P, NT, C], BF16)   # values transposed, [n, nt, c]
    c2_sb = const.tile([DK, M], F16)       # gamma_c * c_attn@p + p on m-slice
    bvb_sb = const.tile([P, C], F32)       # bv broadcast to all 128 partitions
    r_sb = const.tile([P, CCH, M], F32)    # channel-out + xq residual

    # ---- phase 1a: q/pc from xqh (PE starts while the big x DMA runs) ----
    with tc.tile_pool(name='ps0', bufs=2, space='PSUM') as ps0:
        for j in range(M // FREE):
            sl = slice(j * FREE, (j + 1) * FREE)
            pq = ps0.tile([DK, FREE], F32, tag='pq')
            for kc in range(KC):
                nc.tensor.matmul(pq[:], lhsT=wq_sb[:, kc],
                                 rhs=xqh_sb[:, kc, sl],
                                 start=(kc == 0), stop=(kc == KC - 1))
            nc.scalar.activation(q_sb[:, sl], pq[:], ACTF.Identity,
                                 bias=bq_sb[:])
            ppc = ps0.tile([DK, FREE], F32, tag='pq')
            for kc in range(KC):
                nc.tensor.matmul(ppc[:], lhsT=wd_sb[:, kc],
                                 rhs=xqh_sb[:, kc, sl],
                                 start=(kc == 0), stop=(kc == KC - 1))
            nc.scalar.activation(pc_sb[:, sl], ppc[:], ACTF.Identity,
                                 bias=bd_sb[:])

    # ---- phase 1b: everything that needs the full x ----
    with tc.tile_pool(name='xp', bufs=1) as xp, \
         tc.tile_pool(name='ps1', bufs=2, space='PSUM') as ps1, \
         tc.tile_pool(name='ps1s', bufs=2, space='PSUM') as ps1s, \
         tc.tile_pool(name='ps1e', bufs=1, space='PSUM') as ps1e:
        x_sb = xp.tile([P, KC, N], F16)
        x_r = x_d.rearrange("(kc p) n -> p kc n", p=P)
        for nq in range(8):
            qsl = slice(nq * (N // 8), (nq + 1) * (N // 8))
            nc.sync.dma_start(x_sb[:, :, qsl], x_r[:, :, qsl])
        xq_sb = xp.tile([P, KC, M], F32)   # fp32 residual source
        nc.sync.dma_start(xq_sb[:], xq_d.rearrange("(kc p) m -> p kc m", p=P))

        # bv broadcast to [128, C] once
        nc.gpsimd.partition_broadcast(bvb_sb[:], bvrow_sb[:], channels=P)

        # k = Wk@x + bk over the full N
        for j in range(N // FREE):
            sl = slice(j * FREE, (j + 1) * FREE)
            pk = ps1.tile([DK, FREE], F32, tag='pk')
            for kc in range(KC):
                nc.tensor.matmul(pk[:], lhsT=wk_sb[:, kc],
                                 rhs=x_sb[:, kc, sl],
                                 start=(kc == 0), stop=(kc == KC - 1))
            nc.scalar.activation(k_sb[:, sl], pk[:], ACTF.Identity,
                                 bias=bk_sb[:])

        # vT and pT tiles: [n, c] = sum_kc x[kc, n]^T W^T[kc, c]  (+ bias)
        for nt in range(NT):
            nsl = slice(nt * P, (nt + 1) * P)
            pv = ps1.tile([P, C], F32, tag='pv')
            for kc in range(KC):
                nc.tensor.matmul(pv[:], lhsT=x_sb[:, kc, nsl],
                                 rhs=wv_sb[:, kc],
                                 start=(kc == 0), stop=(kc == KC - 1))
            nc.vector.tensor_add(vT_sb[:, nt], in0=pv[:], in1=bvb_sb[:])

            pt_ps = ps1s.tile([P, DK], F32, tag='ptp')
            for kc in range(KC):
                nc.tensor.matmul(pt_ps[:], lhsT=x_sb[:, kc, nsl],
                                 rhs=wd_sb[:, kc],
                                 start=(kc == 0), stop=False)
            nc.tensor.matmul(pt_ps[:], lhsT=ones_row16[:], rhs=bdrow_sb[:],
                             start=False, stop=True)
            nc.vector.tensor_copy(pT_sb[:, nt], pt_ps[:])

        # ---- channel attention (tiny): e = pT^T @ pT, softmax, c2, R ----
        with tc.tile_pool(name='sb2', bufs=2) as sb2:
            e_ps = ps1e.tile([DK, DK], F32, tag='e')
            for nt in range(NT):
                nc.tensor.matmul(e_ps[:], lhsT=pT_sb[:, nt], rhs=pT_sb[:, nt],
                                 start=(nt == 0), stop=(nt == NT - 1))
            # c_attn = softmax(rowmax(e) - e) == exp(rowmin(e) - e) / rowsum
            e_sb = sb2.tile([DK, DK], F32, tag='e')
            nc.vector.tensor_copy(e_sb[:], e_ps[:])
            mn_sb = sb2.tile([DK, 1], F32, tag='mn')
            nc.vector.tensor_reduce(mn_sb[:], e_sb[:], axis=AX.X, op=ALU.min)
            h_sb = sb2.tile([DK, DK], F32, tag='h')
            nc.scalar.activation(h_sb[:], e_sb[:], ACTF.Exp,
                                 bias=mn_sb[:], scale=-1.0)
            zc_sb = sb2.tile([DK, 1], F32, tag='zc')
            nc.vector.tensor_reduce(zc_sb[:], h_sb[:], axis=AX.X, op=ALU.add)
            nc.vector.reciprocal(zc_sb[:], zc_sb[:])
            cat16_sb = sb2.tile([DK, DK], F16, tag='cat16')
            nc.vector.tensor_scalar_mul(cat16_sb[:], in0=h_sb[:],
                                        scalar1=zc_sb[:])
            catT_ps = ps1e.tile([DK, DK], F16, tag='catp')
            nc.tensor.transpose(catT_ps[:], cat16_sb[:], ident16[:])
            catT_sb = sb2.tile([DK, DK], F16, tag='cat')
            nc.vector.tensor_copy(catT_sb[:], catT_ps[:])

            # c2 = gamma_c * (c_attn @ p)[:, m_slice] + pc
            for j in range(M // FREE):
                sl = slice(j * FREE, (j + 1) * FREE)
                co_ps = ps1.tile([DK, FREE], F32, tag='pk')
                nc.tensor.matmul(co_ps[:], lhsT=catT_sb[:], rhs=pc_sb[:, sl],
                                 start=True, stop=True)
                nc.vector.scalar_tensor_tensor(
                    out=c2_sb[:, sl], in0=co_ps[:], scalar=gc_sb[:],
                    in1=pc_sb[:, sl], op0=ALU.mult, op1=ALU.add)

            # R = Wu@c2 + bu + xq  (the whole non-spatial part of the output)
            for mc in range(MCH):
                msl = slice(mc * FREE, (mc + 1) * FREE)
                for cc in range(CCH):
                    wu_ps = ps1.tile([P, FREE], F32, tag='pv')
                    nc.tensor.matmul(wu_ps[:],
                                     lhsT=wu_sb[:, cc * P:(cc + 1) * P],
                                     rhs=c2_sb[:, msl], start=True, stop=True)
                    cob_sb = sb2.tile([P, FREE], F32, tag='cob')
                    nc.scalar.activation(cob_sb[:], wu_ps[:], ACTF.Identity,
                                         bias=bu_sb[:, cc:cc + 1])
                    nc.vector.tensor_add(r_sb[:, cc, msl], in0=cob_sb[:],
                                         in1=xq_sb[:, cc, msl])

    # ---- main loop: E^T -> exp -> U/Z accumulation, one m-chunk at a time ----
    out_r = out_d.rearrange("(kc p) m -> p kc m", p=P)
    with tc.tile_pool(name='upool', bufs=4, space='PSUM') as upool, \
         tc.tile_pool(name='epool', bufs=3, space='PSUM') as epool, \
         tc.tile_pool(name='zpool', bufs=1, space='PSUM') as zpool, \
         tc.tile_pool(name='pt', bufs=6) as ptp, \
         tc.tile_pool(name='ssb', bufs=2) as ssb, \
         tc.tile_pool(name='ot', bufs=4) as otp:
        for mc in range(MCH):
            msl = slice(mc * FREE, (mc + 1) * FREE)
            u_ps = [upool.tile([P, FREE], F32, tag='u', name=f'u{mc}_{i}')
                    for i in range(CCH)]
            s_sb = ssb.tile([P, FREE], BF16, tag='s')
            for nt in range(NT):
                nsl = slice(nt * P, (nt + 1) * P)
                e_t = epool.tile([P, FREE], F32, tag='et')
                nc.tensor.matmul(e_t[:], lhsT=k_sb[:, nsl],
                                 rhs=q_sb[:, msl], start=True, stop=True)
                p_t = ptp.tile([P, FREE], BF16, tag='p')
                nc.scalar.activation(p_t[:], e_t[:], ACTF.Exp)
                if nt == 0:
                    nc.vector.tensor_copy(s_sb[:], p_t[:])
                else:
                    nc.vector.tensor_add(s_sb[:], in0=s_sb[:], in1=p_t[:])
                for cc in range(CCH):
                    nc.tensor.matmul(u_ps[cc][:],
                                     lhsT=vT_sb[:, nt, cc * P:(cc + 1) * P],
                                     rhs=p_t[:],
                                     start=(nt == 0), stop=(nt == NT - 1))
            # Z = colsum(S); Zb = gamma_s / Z broadcast to 128 partitions
            z_ps = zpool.tile([1, FREE], F32, tag='z')
            nc.tensor.matmul(z_ps[:], lhsT=ones_colb[:], rhs=s_sb[:],
                             start=True, stop=True)
            zr_sb = ssb.tile([1, FREE], F32, tag='zr')
            nc.vector.reciprocal_approx_fast(out=zr_sb[:], in_=z_ps[:])
            zrb_sb = ssb.tile([1, FREE], BF16, tag='zrb')
            nc.vector.tensor_scalar_mul(zrb_sb[:], in0=zr_sb[:], scalar1=gs_sb[:])
            zb_ps = zpool.tile([P, FREE], F32, tag='z')
            nc.tensor.matmul(zb_ps[:], lhsT=ones_rowb[:], rhs=zrb_sb[:],
                             start=True, stop=True)
            zb_sb = ssb.tile([P, FREE], F32, tag='zb')
            nc.vector.tensor_copy(zb_sb[:], zb_ps[:])
            # combine: out = U*(gamma_s/Z) + R, then store
            for cc in range(CCH):
                o_sb = otp.tile([P, FREE], F32, tag='o')
                nc.vector.tensor_tensor(o_sb[:], u_ps[cc][:], zb_sb[:], ALU.mult)
                nc.vector.tensor_add(o_sb[:], in0=o_sb[:], in1=r_sb[:, cc, msl])
                nc.sync.dma_start(out_r[:, cc, msl], o_sb[:])

    const_cm.__exit__(None, None, None)


_CACHE = {}


def _get_compiled():
    if 'nc' in _CACHE:
        return _CACHE['nc']
    nc = bacc.Bacc("TRN2", num_devices=NCORES)
    io = {
        'x': nc.dram_tensor('x', [C, N], F16, kind='ExternalInput').ap(),
        'xq': nc.dram_tensor('xq', [C, M], F32, kind='ExternalInput').ap(),
        'xqh': nc.dram_tensor('xqh', [C, M], F16, kind='ExternalInput').ap(),
        'wqT': nc.dram_tensor('wqT', [C, DK], F16, kind='ExternalInput').ap(),
        'wkT': nc.dram_tensor('wkT', [C, DK], F16, kind='ExternalInput').ap(),
        'wvT': nc.dram_tensor('wvT', [C, C], F16, kind='ExternalInput').ap(),
        'wdT': nc.dram_tensor('wdT', [C, DK], F16, kind='ExternalInput').ap(),
        'wuT': nc.dram_tensor('wuT', [DK, C], F16, kind='ExternalInput').ap(),
        'bq': nc.dram_tensor('bq', [DK], F32, kind='ExternalInput').ap(),
        'bk': nc.dram_tensor('bk', [DK], F32, kind='ExternalInput').ap(),
        'bd': nc.dram_tensor('bd', [DK], F32, kind='ExternalInput').ap(),
        'bdh': nc.dram_tensor('bdh', [DK], F16, kind='ExternalInput').ap(),
        'bv': nc.dram_tensor('bv', [C], F32, kind='ExternalInput').ap(),
        'bu': nc.dram_tensor('bu', [C], F32, kind='ExternalInput').ap(),
        'gs': nc.dram_tensor('gs', [1], F32, kind='ExternalInput').ap(),
        'gc': nc.dram_tensor('gc', [1], F32, kind='ExternalInput').ap(),
        'out': nc.dram_tensor('out', [C, M], F32, kind='ExternalOutput').ap(),
    }
    with tile.TileContext(nc) as tc:
        _build_program(tc, io)
    nc.compile()
    _CACHE['nc'] = nc
    return nc


def make_in_maps(x, Wq, bq, Wk, bk, Wv, bv, gamma_s, Wd, bd, Wu, bu, gamma_c):
    """Build the 8 per-core input dicts from the full problem inputs."""
    f32 = lambda a: np.ascontiguousarray(np.asarray(a, dtype=np.float32))
    f16 = lambda a: np.ascontiguousarray(np.asarray(a, dtype=np.float32)
                                         .astype(np.float16))
    x = f32(x).reshape(B, C, N)
    shared = {
        'wqT': f16(np.asarray(Wq).T), 'wkT': f16(np.asarray(Wk).T),
        'wvT': f16(np.asarray(Wv).T), 'wdT': f16(np.asarray(Wd).T),
        'wuT': f16(np.asarray(Wu).T),
        'bq': f32(bq), 'bk': f32(bk), 'bd': f32(bd), 'bdh': f16(bd),
        'bv': f32(bv), 'bu': f32(bu), 'gs': f32(gamma_s), 'gc': f32(gamma_c),
    }
    in_maps = []
    for core in range(NCORES):
        b, h = divmod(core, 2)
        xq = x[b][:, h * M:(h + 1) * M]
        in_maps.append({
            'x': f16(x[b]),
            'xq': f32(xq),
            'xqh': f16(xq),
            **shared,
        })
    return in_maps


def assemble_out(results):
    """Stitch the 8 per-core [C, M] outputs back to [B, C, W, H]."""
    full = np.empty((B, C, N), np.float32)
    for core, res in enumerate(results):
        b, h = divmod(core, 2)
        full[b][:, h * M:(h + 1) * M] = res['out']
    return full.reshape(B, C, WIDTH, HEIGHT)


def kernel(**inputs):
    nc = _get_compiled()
    in_maps = make_in_maps(**inputs)
    res = bass_utils.run_bass_kernel_spmd(nc, in_maps, core_ids=list(range(NCORES)))
    return assemble_out(res.results)
